# revision 2
# baseline (speedup 1.0000x reference)
"""Multi-head attention forward (B=2, S=2048, E=1024, H=16, D=64) on 8 TRN2
NeuronCores, tensor-parallel across heads (2 heads/core). v2.

All data bf16 (fp32 PSUM accumulation). Per core:
- QKV^T projection with W stationary / X^T moving for Q^T,K^T; V is computed
  in natural [keys, d] layout directly (X^T chunk stationary, Wv moving), so
  no PE transposes are needed.
- v1 layout per (b, kc): [V_h0 (64) | ones | V_h1 (64) | ones]; the PV
  stationary of head h is the contiguous 65-col window at h*65, so each
  head's psum comes out as [attn rows 0..63, denominator row 64].
- Scores in the S^T orientation; a supergroup [128, 2 heads, 512] fp32
  (one kc, both heads) fits 2 banks and one ACT exp covers 1024 cols.
- Emission is wavefront-ordered: attention(b0,qb0) starts after the first
  QKV row block; QKV of b1 is emitted as PE filler inside ACT-bound
  attention(b0); y rows DMA out per 128-row block in bf16.
Host sums the 8 partial y's in fp32 and adds the output bias.
"""

import os
from contextlib import ExitStack

import numpy as np

import concourse.bass as bass
import concourse.mybir as mybir
import concourse.tile as tile
from concourse import bacc

# ---- problem constants (hardcoded per contract) ----
B, S, E, H, D = 2, 2048, 1024, 16, 64
P = 128                      # partitions
R = B * S                    # 4096 flattened rows
KO = E // P                  # 8 contraction chunks over E
NKC = S // P                 # 16 key chunks per sequence
HC = 2                       # heads per core
NCORES = 8
RB = 512                     # row block for the QKV projection
QB = 512                     # query block
NSG = NKC                    # supergroups (= key chunks) per (b, qb)
NQB = S // QB                # 4
NRB = S // RB                # 4 row blocks per batch

BF16 = mybir.dt.bfloat16
FP32 = mybir.dt.float32
EXP = mybir.ActivationFunctionType.Exp

ES_BUFS = int(os.environ.get("MHA_ES_BUFS", "4"))
XT_BUFS = int(os.environ.get("MHA_XT_BUFS", "3"))


def build_kernel(tc, xt, wqkv, bqkv, bqv_row, wout, y, ctx):
    nc = tc.nc

    const = ctx.enter_context(tc.tile_pool(name="const", bufs=1))
    ps_sc = ctx.enter_context(tc.tile_pool(name="ps_sc", bufs=2, space="PSUM"))
    ps_pa = ctx.enter_context(tc.tile_pool(name="ps_pa", bufs=2, space="PSUM"))
    ps_q = ctx.enter_context(tc.tile_pool(name="ps_q", bufs=2, space="PSUM"))

    wq_sb = const.tile([P, KO, 3 * P], BF16)
    wq_r = wqkv.rearrange("(ko p) m -> p ko m", p=P)
    for ko in range(KO):
        nc.sync.dma_start(wq_sb[:, ko, :], wq_r[:, ko, :])
    bq_sb = const.tile([P, 2], FP32)          # Q,K biases per-partition
    nc.sync.dma_start(bq_sb, bqkv.rearrange("(m p) -> p m", p=P))
    bv_row = const.tile([1, P], FP32)         # V bias as a row
    nc.sync.dma_start(bv_row, bqv_row)
    wo_sb = const.tile([P, E], BF16)
    nc.sync.dma_start(wo_sb, wout)

    bv_bc = const.tile([P, 1, P], FP32)       # V bias broadcast to all parts
    nc.gpsimd.partition_broadcast(bv_bc[:, 0, :], bv_row)

    qt = const.tile([P, B, S], BF16)          # Q^T  [2h*64, b, s]
    kt = const.tile([P, B, S], BF16)          # K^T
    v1 = const.tile([P, B, NKC, 2 * D + 2], BF16)  # [V_h0|1|V_h1|1]
    attnT = const.tile([P, B, S], BF16)

    ones_col = const.tile([P, 1], FP32)
    nc.vector.memset(ones_col, 1.0)
    nc.vector.tensor_copy(v1[:, :, :, D:D + 1],
                          ones_col.to_broadcast((P, B, NKC, 1)))
    nc.vector.tensor_copy(v1[:, :, :, 2 * D + 1:2 * D + 2],
                          ones_col.to_broadcast((P, B, NKC, 1)))
    # trigger the exp table load early (overlaps the QKV prologue)
    dum = const.tile([1, 1], FP32)
    nc.scalar.activation(dum, ones_col[:1, :1], EXP, scale=1.0)

    xt_pool = ctx.enter_context(tc.tile_pool(name="xtp", bufs=XT_BUFS))
    exps_pool = ctx.enter_context(tc.tile_pool(name="exps", bufs=ES_BUFS))
    rc_pool = ctx.enter_context(tc.tile_pool(name="rc", bufs=2))
    bc_pool = ctx.enter_context(tc.tile_pool(name="bc", bufs=2))
    y_pool = ctx.enter_context(tc.tile_pool(name="yp", bufs=3))

    xt_r = xt.rearrange("(ko p) r -> p ko r", p=P)

    xt_tiles = {}

    def xt_dma(b, rbi):
        rb = b * NRB + rbi
        t = xt_pool.tile([P, KO, RB], BF16, tag="xt", name=f"xt_{rb}")
        for ko in range(KO):
            nc.sync.dma_start(t[:, ko, :], xt_r[:, ko, rb * RB:(rb + 1) * RB])
        xt_tiles[rb] = t

    def qk_pass(b, rbi, m):
        """m: 0=Q, 1=K. Produces (qt|kt)[:, b, rbi*RB:...] via W-stationary."""
        rb = b * NRB + rbi
        xt_t = xt_tiles[rb]
        col = rbi * RB
        pst = ps_q.tile([P, RB], FP32, tag="pq", name=f"ps_qk_{rb}_{m}")
        for ko in range(KO):
            nc.tensor.matmul(pst, wq_sb[:, ko, m * P:(m + 1) * P],
                             xt_t[:, ko, :],
                             start=(ko == 0), stop=(ko == KO - 1))
        dest = qt if m == 0 else kt
        nc.vector.tensor_scalar_add(dest[:, b, col:col + RB], pst,
                                    bq_sb[:, m:m + 1])

    def v_pass(b, rbi):
        """V natural via X^T-chunk stationary: psum [128 rows, 128 (2 heads)]
        per 128-row chunk; 4 chunks share one psum bank."""
        rb = b * NRB + rbi
        xt_t = xt_tiles[rb]
        pst = ps_q.tile([P, NKC // NRB, P], FP32, tag="pq", name=f"ps_v_{rb}")
        for c in range(NKC // NRB):
            for ko in range(KO):
                nc.tensor.matmul(
                    pst[:, c, :], xt_t[:, ko, c * P:(c + 1) * P],
                    wq_sb[:, ko, 2 * P:3 * P],
                    start=(ko == 0), stop=(ko == KO - 1),
                    skip_group_check=True)
        kc0 = rbi * (NKC // NRB)
        # h0 -> v1 cols 0..63, h1 -> v1 cols 65..128 (col 64 is the ones col)
        nc.vector.tensor_tensor(
            v1[:, b, kc0:kc0 + 4, 0:D], pst[:, :, 0:D],
            bv_bc[:, :, 0:D].to_broadcast((P, 4, D)), mybir.AluOpType.add)
        nc.vector.tensor_tensor(
            v1[:, b, kc0:kc0 + 4, D + 1:2 * D + 1], pst[:, :, D:2 * D],
            bv_bc[:, :, D:2 * D].to_broadcast((P, 4, D)), mybir.AluOpType.add)

    pa = {}

    def att_sg(b, qb, kc):
        """One supergroup: 2 score MMs (row-group interleaved) into a 2-bank
        fp32 psum tile, one exp over 1024 cols, 2 PV MMs into pa."""
        pst = ps_sc.tile([P, HC, QB], FP32, tag="sc",
                         name=f"sc_{b}_{qb}_{kc}")
        est = exps_pool.tile([P, HC, QB], BF16, tag="es",
                             name=f"es_{b}_{qb}_{kc}")
        for h in range(HC):
            nc.tensor.matmul(
                pst[:, h, :],
                kt[h * D:(h + 1) * D, b, kc * P:(kc + 1) * P],
                qt[h * D:(h + 1) * D, b, qb * QB:(qb + 1) * QB],
                start=True, stop=True)
        nc.scalar.activation(est, pst, EXP, scale=0.125)
        for h in range(HC):
            nc.tensor.matmul(
                pa[h][:D + 1, :], v1[:, b, kc, h * (D + 1):(h + 1) * (D + 1)],
                est[:, h, :],
                start=(kc == 0), stop=(kc == NKC - 1),
                skip_group_check=True)

    def att_qb(b, qb, fillers=()):
        """Full attention for one (b, qb) query block, interleaving filler
        emission (QKV passes of the other batch) between supergroups."""
        for h in range(HC):
            pa[h] = ps_pa.tile([P, QB], FP32, tag="pa",
                               name=f"pa_{b}_{qb}_{h}")
        fl = list(fillers)
        for sg in range(NSG):
            att_sg(b, qb, sg)
            if sg % 4 == 3 and fl:
                fl.pop(0)()
        for f in fl:
            f()
        att_qb_tail(b, qb)

    def att_qb_tail(b, qb):
        # normalize: both heads have attn rows 0..63, denominator row 64
        for h in range(HC):
            rc = rc_pool.tile([1, QB], FP32, tag="rc", name=f"rc_{b}_{qb}_{h}")
            nc.vector.reciprocal(rc, pa[h][D:D + 1, :])
            bc = bc_pool.tile([D, QB], FP32, tag="bc", name=f"bc_{b}_{qb}_{h}")
            nc.gpsimd.partition_broadcast(bc, rc)
            nc.vector.tensor_tensor(
                attnT[h * D:(h + 1) * D, b, qb * QB:(qb + 1) * QB],
                pa[h][:D, :], bc, mybir.AluOpType.mult)
        for qc in range(QB // P):
            q0 = qb * QB + qc * P
            yt = y_pool.tile([P, E], BF16, tag="yt", name=f"yt_{b}_{qb}_{qc}")
            for nh in range(2):
                pst = ps_q.tile([P, 512], FP32, tag="pq",
                                name=f"ps_y_{b}_{qb}_{qc}_{nh}")
                nc.tensor.matmul(
                    pst, attnT[:, b, q0:q0 + P],
                    wo_sb[:, nh * 512:(nh + 1) * 512],
                    start=True, stop=True)
                nc.vector.tensor_copy(yt[:, nh * 512:(nh + 1) * 512], pst)
            nc.sync.dma_start(y[b * S + q0: b * S + q0 + P, :], yt)

    # ---- emission ----
    # prologue: b0 row blocks wavefront with attention(b0, qb0)
    xt_dma(0, 0)
    xt_dma(0, 1)
    for h in range(HC):
        pa[h] = ps_pa.tile([P, QB], FP32, tag="pa", name=f"pa_0_0_{h}")
    for rbi in range(NRB):
        if rbi + 2 < NRB:
            xt_dma(0, rbi + 2)
        qk_pass(0, rbi, 1)          # K
        v_pass(0, rbi)              # V
        qk_pass(0, rbi, 0)          # Q
        for c in range(4):
            att_sg(0, 0, 4 * rbi + c)
    att_qb_tail(0, 0)

    # b0 qb1..3 with b1's QKV as filler
    b1_fill = []
    for rbi in range(NRB):
        b1_fill.append(lambda rbi=rbi: xt_dma(1, rbi))
        b1_fill.append(lambda rbi=rbi: qk_pass(1, rbi, 1))
        b1_fill.append(lambda rbi=rbi: v_pass(1, rbi))
        b1_fill.append(lambda rbi=rbi: qk_pass(1, rbi, 0))
    nfl = (len(b1_fill) + 2) // 3
    for qb in range(1, NQB):
        take, b1_fill = b1_fill[:nfl], b1_fill[nfl:]
        att_qb(0, qb, take)

    att_qb(1, 0, b1_fill)
    att_qb(1, 1)
    att_qb(1, 2)
    att_qb(1, 3)


def build_nc(mm_mode="bf16", reps=1):
    nc = bacc.Bacc("TRN2", target_bir_lowering=False, debug=False)
    xt = nc.dram_tensor("xt", [E, R], BF16, kind="ExternalInput").ap()
    wqkv = nc.dram_tensor("wqkv", [E, 3 * P], BF16, kind="ExternalInput").ap()
    bqkv = nc.dram_tensor("bqkv", [2 * P], FP32, kind="ExternalInput").ap()
    bqv = nc.dram_tensor("bqv", [1, P], FP32, kind="ExternalInput").ap()
    wout = nc.dram_tensor("wout", [P, E], BF16, kind="ExternalInput").ap()
    y = nc.dram_tensor("y", [R, E], BF16, kind="ExternalOutput").ap()
    with tile.TileContext(nc) as tc:
        for _ in range(reps):
            with ExitStack() as ctx:
                build_kernel(tc, xt, wqkv, bqkv, bqv, wout, y, ctx)
    nc.compile()
    return nc


def shard_inputs(input_tensor, qkv_w, qkv_b, out_w, mm_mode="bf16"):
    np_bf16 = mybir.dt.np(BF16)

    def prep(a):
        return np.ascontiguousarray(a).astype(np_bf16)

    X = np.asarray(input_tensor, np.float32).reshape(R, E)
    XT = prep(X.T)
    qkv_w = np.asarray(qkv_w, np.float32)
    qkv_b = np.asarray(qkv_b, np.float32)
    out_w = np.asarray(out_w, np.float32)
    in_maps = []
    for c in range(NCORES):
        sl = slice(c * P, (c + 1) * P)
        wq = np.concatenate(
            [qkv_w[:, sl], qkv_w[:, E + c * P:E + (c + 1) * P],
             qkv_w[:, 2 * E + c * P:2 * E + (c + 1) * P]], axis=1)
        bq = np.concatenate(
            [qkv_b[sl], qkv_b[E + c * P:E + (c + 1) * P]])
        bv = qkv_b[2 * E + c * P:2 * E + (c + 1) * P].reshape(1, P)
        in_maps.append({
            "xt": XT,
            "wqkv": prep(wq),
            "bqkv": np.ascontiguousarray(bq, np.float32),
            "bqv": np.ascontiguousarray(bv, np.float32),
            "wout": prep(out_w[sl, :]),
        })
    return in_maps


_NC_CACHE = {}
MM_MODE = "bf16"


def _get_nc(mm_mode="bf16"):
    if mm_mode not in _NC_CACHE:
        _NC_CACHE[mm_mode] = build_nc(mm_mode)
    return _NC_CACHE[mm_mode]


LAST_RESULT = None


def kernel(input_tensor, qkv_w, qkv_b, out_w, out_b):
    global LAST_RESULT
    from concourse import bass_utils
    nc = _get_nc(MM_MODE)
    in_maps = shard_inputs(input_tensor, qkv_w, qkv_b, out_w)
    res = bass_utils.run_bass_kernel_spmd(
        nc, in_maps, core_ids=list(range(NCORES)))
    LAST_RESULT = res
    out = np.zeros((R, E), np.float32)
    for r in res.results:
        out += np.asarray(r["y"], np.float32)
    out += np.asarray(out_b, np.float32)
    return out.reshape(B, S, E)


# revision 3
# speedup vs baseline: 1.0762x; 1.0762x over previous
"""Multi-head attention forward (B=2, S=2048, E=1024, H=16, D=64) on 8 TRN2
NeuronCores, tensor-parallel across heads (2 heads/core). v2.

All data bf16 (fp32 PSUM accumulation). Per core:
- QKV^T projection with W stationary / X^T moving for Q^T,K^T; V is computed
  in natural [keys, d] layout directly (X^T chunk stationary, Wv moving), so
  no PE transposes are needed.
- v1 layout per (b, kc): [V_h0 (64) | ones | V_h1 (64) | ones]; the PV
  stationary of head h is the contiguous 65-col window at h*65, so each
  head's psum comes out as [attn rows 0..63, denominator row 64].
- Scores in the S^T orientation; a supergroup [128, 2 heads, 512] fp32
  (one kc, both heads) fits 2 banks and one ACT exp covers 1024 cols.
- Emission is wavefront-ordered: attention(b0,qb0) starts after the first
  QKV row block; QKV of b1 is emitted as PE filler inside ACT-bound
  attention(b0); y rows DMA out per 128-row block in bf16.
Host sums the 8 partial y's in fp32 and adds the output bias.
"""

import os
from contextlib import ExitStack

import numpy as np

import concourse.bass as bass
import concourse.mybir as mybir
import concourse.tile as tile
from concourse import bacc

# ---- problem constants (hardcoded per contract) ----
B, S, E, H, D = 2, 2048, 1024, 16, 64
P = 128                      # partitions
R = B * S                    # 4096 flattened rows
KO = E // P                  # 8 contraction chunks over E
NKC = S // P                 # 16 key chunks per sequence
HC = 2                       # heads per core
NCORES = 8
RB = 512                     # row block for the QKV projection
QB = 512                     # query block
NSG = NKC                    # supergroups (= key chunks) per (b, qb)
NQB = S // QB                # 4
NRB = S // RB                # 4 row blocks per batch

BF16 = mybir.dt.bfloat16
FP32 = mybir.dt.float32
EXP = mybir.ActivationFunctionType.Exp

ES_BUFS = int(os.environ.get("MHA_ES_BUFS", "4"))
XT_BUFS = int(os.environ.get("MHA_XT_BUFS", "3"))


def build_kernel(tc, xt, wqkv, bqkv, bqv_row, wout, y, ctx):
    nc = tc.nc

    const = ctx.enter_context(tc.tile_pool(name="const", bufs=1))
    ps_sc = ctx.enter_context(tc.tile_pool(name="ps_sc", bufs=2, space="PSUM"))
    ps_pa = ctx.enter_context(tc.tile_pool(name="ps_pa", bufs=2, space="PSUM"))
    ps_q = ctx.enter_context(tc.tile_pool(name="ps_q", bufs=2, space="PSUM"))

    wq_sb = const.tile([P, KO, 3 * P], BF16)
    wq_r = wqkv.rearrange("(ko p) m -> p ko m", p=P)
    for ko in range(KO):
        nc.sync.dma_start(wq_sb[:, ko, :], wq_r[:, ko, :])
    bq_sb = const.tile([P, 2], FP32)          # Q,K biases per-partition
    nc.sync.dma_start(bq_sb, bqkv.rearrange("(m p) -> p m", p=P))
    bv_row = const.tile([1, P], FP32)         # V bias as a row
    nc.sync.dma_start(bv_row, bqv_row)
    wo_sb = const.tile([P, E], BF16)
    nc.sync.dma_start(wo_sb, wout)

    bv_bc = const.tile([P, 1, P], FP32)       # V bias broadcast to all parts
    nc.gpsimd.partition_broadcast(bv_bc[:, 0, :], bv_row)

    qt = const.tile([P, B, S], BF16)          # Q^T  [2h*64, b, s]
    kt = const.tile([P, B, S], BF16)          # K^T
    v1 = const.tile([P, B, NKC, 2 * D + 2], BF16)  # [V_h0|1|V_h1|1]
    attnT = const.tile([P, B, S], BF16)

    ones_col = const.tile([P, 1], FP32)
    nc.vector.memset(ones_col, 1.0)
    nc.vector.tensor_copy(v1[:, :, :, D:D + 1],
                          ones_col.to_broadcast((P, B, NKC, 1)))
    nc.vector.tensor_copy(v1[:, :, :, 2 * D + 1:2 * D + 2],
                          ones_col.to_broadcast((P, B, NKC, 1)))
    # trigger the exp table load early (overlaps the QKV prologue)
    dum = const.tile([1, 1], FP32)
    nc.scalar.activation(dum, ones_col[:1, :1], EXP, scale=1.0)

    xt_pool = ctx.enter_context(tc.tile_pool(name="xtp", bufs=XT_BUFS))
    exps_pool = ctx.enter_context(tc.tile_pool(name="exps", bufs=ES_BUFS))
    rc_pool = ctx.enter_context(tc.tile_pool(name="rc", bufs=2))
    bc_pool = ctx.enter_context(tc.tile_pool(name="bc", bufs=2))
    y_pool = ctx.enter_context(tc.tile_pool(name="yp", bufs=3))

    xt_r = xt.rearrange("(ko p) r -> p ko r", p=P)

    xt_tiles = {}

    def xt_dma(b, rbi):
        rb = b * NRB + rbi
        t = xt_pool.tile([P, KO, RB], BF16, tag="xt", name=f"xt_{rb}")
        for ko in range(KO):
            nc.sync.dma_start(t[:, ko, :], xt_r[:, ko, rb * RB:(rb + 1) * RB])
        xt_tiles[rb] = t

    def qk_pass(b, rbi, m):
        """m: 0=Q, 1=K. Produces (qt|kt)[:, b, rbi*RB:...] via W-stationary."""
        rb = b * NRB + rbi
        xt_t = xt_tiles[rb]
        col = rbi * RB
        pst = ps_q.tile([P, RB], FP32, tag="pq", name=f"ps_qk_{rb}_{m}")
        for ko in range(KO):
            nc.tensor.matmul(pst, wq_sb[:, ko, m * P:(m + 1) * P],
                             xt_t[:, ko, :],
                             start=(ko == 0), stop=(ko == KO - 1))
        dest = qt if m == 0 else kt
        nc.vector.tensor_scalar_add(dest[:, b, col:col + RB], pst,
                                    bq_sb[:, m:m + 1])

    def v_pass(b, rbi):
        """V natural via X^T-chunk stationary: psum [128 rows, 128 (2 heads)]
        per 128-row chunk; 4 chunks share one psum bank."""
        rb = b * NRB + rbi
        xt_t = xt_tiles[rb]
        pst = ps_q.tile([P, NKC // NRB, P], FP32, tag="pq", name=f"ps_v_{rb}")
        for c in range(NKC // NRB):
            for ko in range(KO):
                nc.tensor.matmul(
                    pst[:, c, :], xt_t[:, ko, c * P:(c + 1) * P],
                    wq_sb[:, ko, 2 * P:3 * P],
                    start=(ko == 0), stop=(ko == KO - 1),
                    skip_group_check=True)
        kc0 = rbi * (NKC // NRB)
        # h0 -> v1 cols 0..63, h1 -> v1 cols 65..128 (col 64 is the ones col)
        nc.vector.tensor_tensor(
            v1[:, b, kc0:kc0 + 4, 0:D], pst[:, :, 0:D],
            bv_bc[:, :, 0:D].to_broadcast((P, 4, D)), mybir.AluOpType.add)
        nc.vector.tensor_tensor(
            v1[:, b, kc0:kc0 + 4, D + 1:2 * D + 1], pst[:, :, D:2 * D],
            bv_bc[:, :, D:2 * D].to_broadcast((P, 4, D)), mybir.AluOpType.add)

    pa = {}

    def att_sg(b, qb, kc):
        """One supergroup: 2 score MMs (row-group interleaved) into a 2-bank
        fp32 psum tile, one exp over 1024 cols, 2 PV MMs into pa."""
        pst = ps_sc.tile([P, HC, QB], FP32, tag="sc",
                         name=f"sc_{b}_{qb}_{kc}")
        est = exps_pool.tile([P, HC, QB], BF16, tag="es",
                             name=f"es_{b}_{qb}_{kc}")
        with tc.high_priority(offset=400):
            for h in range(HC):
                nc.tensor.matmul(
                    pst[:, h, :],
                    kt[h * D:(h + 1) * D, b, kc * P:(kc + 1) * P],
                    qt[h * D:(h + 1) * D, b, qb * QB:(qb + 1) * QB],
                    start=True, stop=True)
            nc.scalar.activation(est, pst, EXP, scale=0.125)
        for h in range(HC):
            nc.tensor.matmul(
                pa[h][:D + 1, :], v1[:, b, kc, h * (D + 1):(h + 1) * (D + 1)],
                est[:, h, :],
                start=(kc == 0), stop=(kc == NKC - 1),
                skip_group_check=True)

    deferred = []

    def att_qb(b, qb, fillers=(), defer=False):
        """Full attention for one (b, qb) query block, interleaving filler
        emission (QKV passes of the other batch) between supergroups."""
        for h in range(HC):
            pa[h] = ps_pa.tile([P, QB], FP32, tag="pa",
                               name=f"pa_{b}_{qb}_{h}")
        fl = list(fillers)
        for sg in range(NSG):
            att_sg(b, qb, sg)
            if sg % 4 == 3 and fl:
                fl.pop(0)()
        for f in fl:
            f()
        att_qb_tail(b, qb, defer=defer)

    def normalize(b, qb):
        # normalize: both heads have attn rows 0..63, denominator row 64
        for h in range(HC):
            rc = rc_pool.tile([1, QB], FP32, tag="rc", name=f"rc_{b}_{qb}_{h}")
            nc.vector.reciprocal(rc, pa[h][D:D + 1, :])
            bc = bc_pool.tile([D, QB], FP32, tag="bc", name=f"bc_{b}_{qb}_{h}")
            nc.gpsimd.partition_broadcast(bc, rc)
            nc.vector.tensor_tensor(
                attnT[h * D:(h + 1) * D, b, qb * QB:(qb + 1) * QB],
                pa[h][:D, :], bc, mybir.AluOpType.mult)

    def outproj(b, qb, qc):
        q0 = qb * QB + qc * P
        yt = y_pool.tile([P, E], BF16, tag="yt", name=f"yt_{b}_{qb}_{qc}")
        for nh in range(2):
            pst = ps_q.tile([P, 512], FP32, tag="pq",
                            name=f"ps_y_{b}_{qb}_{qc}_{nh}")
            nc.tensor.matmul(
                pst, attnT[:, b, q0:q0 + P],
                wo_sb[:, nh * 512:(nh + 1) * 512],
                start=True, stop=True)
            nc.vector.tensor_copy(yt[:, nh * 512:(nh + 1) * 512], pst)
        nc.sync.dma_start(y[b * S + q0: b * S + q0 + P, :], yt)

    def att_qb_tail(b, qb, defer=False):
        normalize(b, qb)
        if defer:
            for qc in range(QB // P):
                deferred.append(lambda b=b, qb=qb, qc=qc: outproj(b, qb, qc))
        else:
            for qc in range(QB // P):
                outproj(b, qb, qc)

    # ---- emission ----
    # prologue: b0 row blocks wavefront with attention(b0, qb0)
    xt_dma(0, 0)
    xt_dma(0, 1)
    for h in range(HC):
        pa[h] = ps_pa.tile([P, QB], FP32, tag="pa", name=f"pa_0_0_{h}")
    for rbi in range(NRB):
        if rbi + 2 < NRB:
            xt_dma(0, rbi + 2)
        qk_pass(0, rbi, 1)          # K
        v_pass(0, rbi)              # V
        qk_pass(0, rbi, 0)          # Q
        for c in range(4):
            att_sg(0, 0, 4 * rbi + c)
    att_qb_tail(0, 0, defer=True)

    # b0 qb1..3 with b1's QKV as filler
    b1_fill = []
    for rbi in range(NRB):
        b1_fill.append(lambda rbi=rbi: xt_dma(1, rbi))
        b1_fill.append(lambda rbi=rbi: qk_pass(1, rbi, 1))
        b1_fill.append(lambda rbi=rbi: v_pass(1, rbi))
        b1_fill.append(lambda rbi=rbi: qk_pass(1, rbi, 0))
    nfl = (len(b1_fill) + 2) // 3
    for qb in range(1, NQB):
        take, b1_fill = b1_fill[:nfl], b1_fill[nfl:]
        att_qb(0, qb, take, defer=True)

    # b0's 16 deferred out-projections spread across b1's ACT-bound phase
    dq = list(b1_fill)
    for qb in range(NQB):
        dq += deferred[qb * 4:(qb + 1) * 4]
    n = (len(dq) + NQB - 1) // NQB
    att_qb(1, 0, dq[:n])
    att_qb(1, 1, dq[n:2 * n])
    att_qb(1, 2, dq[2 * n:3 * n])
    att_qb(1, 3, dq[3 * n:])


def build_nc(mm_mode="bf16", reps=1):
    nc = bacc.Bacc("TRN2", target_bir_lowering=False, debug=False)
    xt = nc.dram_tensor("xt", [E, R], BF16, kind="ExternalInput").ap()
    wqkv = nc.dram_tensor("wqkv", [E, 3 * P], BF16, kind="ExternalInput").ap()
    bqkv = nc.dram_tensor("bqkv", [2 * P], FP32, kind="ExternalInput").ap()
    bqv = nc.dram_tensor("bqv", [1, P], FP32, kind="ExternalInput").ap()
    wout = nc.dram_tensor("wout", [P, E], BF16, kind="ExternalInput").ap()
    y = nc.dram_tensor("y", [R, E], BF16, kind="ExternalOutput").ap()
    with tile.TileContext(nc) as tc:
        for _ in range(reps):
            with ExitStack() as ctx:
                build_kernel(tc, xt, wqkv, bqkv, bqv, wout, y, ctx)
    nc.compile()
    return nc


def shard_inputs(input_tensor, qkv_w, qkv_b, out_w, mm_mode="bf16"):
    np_bf16 = mybir.dt.np(BF16)

    def prep(a):
        return np.ascontiguousarray(a).astype(np_bf16)

    X = np.asarray(input_tensor, np.float32).reshape(R, E)
    XT = prep(X.T)
    qkv_w = np.asarray(qkv_w, np.float32)
    qkv_b = np.asarray(qkv_b, np.float32)
    out_w = np.asarray(out_w, np.float32)
    in_maps = []
    for c in range(NCORES):
        sl = slice(c * P, (c + 1) * P)
        wq = np.concatenate(
            [qkv_w[:, sl], qkv_w[:, E + c * P:E + (c + 1) * P],
             qkv_w[:, 2 * E + c * P:2 * E + (c + 1) * P]], axis=1)
        bq = np.concatenate(
            [qkv_b[sl], qkv_b[E + c * P:E + (c + 1) * P]])
        bv = qkv_b[2 * E + c * P:2 * E + (c + 1) * P].reshape(1, P)
        in_maps.append({
            "xt": XT,
            "wqkv": prep(wq),
            "bqkv": np.ascontiguousarray(bq, np.float32),
            "bqv": np.ascontiguousarray(bv, np.float32),
            "wout": prep(out_w[sl, :]),
        })
    return in_maps


_NC_CACHE = {}
MM_MODE = "bf16"


def _get_nc(mm_mode="bf16"):
    if mm_mode not in _NC_CACHE:
        _NC_CACHE[mm_mode] = build_nc(mm_mode)
    return _NC_CACHE[mm_mode]


LAST_RESULT = None


def kernel(input_tensor, qkv_w, qkv_b, out_w, out_b):
    global LAST_RESULT
    from concourse import bass_utils
    nc = _get_nc(MM_MODE)
    in_maps = shard_inputs(input_tensor, qkv_w, qkv_b, out_w)
    res = bass_utils.run_bass_kernel_spmd(
        nc, in_maps, core_ids=list(range(NCORES)))
    LAST_RESULT = res
    out = np.zeros((R, E), np.float32)
    for r in res.results:
        out += np.asarray(r["y"], np.float32)
    out += np.asarray(out_b, np.float32)
    return out.reshape(B, S, E)
